# revision 1
# baseline (speedup 1.0000x reference)
"""Trainium2 Bass kernel for a MAGNA-KG message-passing layer.

Problem: N=50000 nodes, E=800000 edges, R=500 relations, D=256 dims,
H=8 heads, 3 PPR hops.  SPMD across 8 NeuronCores.

Sharding (edge parallelism per the hint):
  * nodes range-sharded: core c owns rows [c*NB, (c+1)*NB), NB=6272
  * edges sharded by owner of dst; within a core edges are grouped by dst
    block (128 nodes), then by src parity (int16 gather-index workaround),
    padded to multiples of 128 -> a fixed chunk structure baked into the
    SPMD-identical instruction stream (per-block chunk counts are maxed
    over cores at build time)
  * per hop: dma_gather of src rows from a replicated bf16 feature table,
    one broadcast multiply by the per-edge attention weight, segment-sum
    via 0/1 indicator matmuls accumulating per dst block in PSUM, blend
    with alpha*feat0, AllGather the new table
  * attention: one-time gathers of eh[src], et[dst], er[eid] from
    256B-padded score tables; ex = exp(leakyrelu(eh+et+er)); denominator
    via the same indicator matmuls; (1-alpha)/den folded into the
    per-dst-block output scale
  * feature columns are permuted to (d-major, head-minor) so per-edge
    head scales broadcast with an innermost step-1 AP

kernel(**inputs) takes FULL inputs and returns the FULL [N, 256] output.
"""

import numpy as np
import ml_dtypes

import concourse.bacc as bacc
import concourse.bass as bass
import concourse.mybir as mybir
from concourse import tile

F32 = mybir.dt.float32
BF16 = mybir.dt.bfloat16
I16 = mybir.dt.int16
AX = mybir.AxisListType
ALU = mybir.AluOpType
ACTF = mybir.ActivationFunctionType

BF = ml_dtypes.bfloat16

NEG_BIG = -1.0e9


class Cfg:
    def __init__(self, N=50000, E=800000, R=500, P=8, HOPS=3,
                 ALPHA=0.15, SLOPE=0.2, EPS=1e-5):
        self.N, self.E, self.R, self.P = N, E, R, P
        self.D, self.H, self.AD = 256, 8, 32
        self.HOPS, self.ALPHA, self.SLOPE, self.EPS = HOPS, ALPHA, SLOPE, EPS
        # blocks of 128 dst nodes per core
        self.B = -(-N // (P * 128))          # ceil
        self.NB = self.B * 128               # nodes per core (padded)
        self.NPAD = P * self.NB              # padded global node count
        self.RPAD = 512                      # relation table rows (>= R+1)
        self.R_SENT = R                      # sentinel row in er table
        assert self.RPAD >= R + 1


def _cdiv(a, b):
    return -(-a // b)


# --------------------------------------------------------------------------
# host-side planning: edge grouping, gather indices, indicator matrices
# --------------------------------------------------------------------------

class Plan:
    pass


def plan_edges(cfg, src, dst, eids):
    """Group edges per core by (dst block, src parity); compute shared chunk
    counts; build per-core index/indicator tensors."""
    P, B, NB = cfg.P, cfg.B, cfg.NB
    src = np.asarray(src).astype(np.int64)
    dst = np.asarray(dst).astype(np.int64)
    eids = np.asarray(eids).astype(np.int64)

    core_of = dst // NB
    per_core = []
    cnts = np.zeros((P, B, 2), np.int64)
    for c in range(P):
        m = core_of == c
        s, d, r = src[m], dst[m], eids[m]
        blk = (d - c * NB) // 128
        par = s & 1
        order = np.lexsort((par, blk))
        s, d, r, blk, par = s[order], d[order], r[order], blk[order], par[order]
        grp = blk * 2 + par
        cnt = np.bincount(grp, minlength=B * 2).reshape(B, 2)
        cnts[c] = cnt
        per_core.append((s, d, r, grp, cnt))

    # shared chunk counts per (block, parity): max over cores, >= 1
    K = np.maximum(_cdiv(cnts, 128).max(axis=0), 1)      # [B, 2]
    Kb = K.sum(axis=1)                                   # [B]
    CO = np.concatenate([[0], np.cumsum(Kb)])            # chunk offset per block
    TOTCH = int(CO[-1])
    TOT = TOTCH * 128

    # padded start position (in edge slots) of each (b, g) group
    gstart = np.zeros((B, 2), np.int64)
    gstart[:, 0] = CO[:-1] * 128
    gstart[:, 1] = CO[:-1] * 128 + K[:, 0] * 128

    pl = Plan()
    pl.K, pl.Kb, pl.CO, pl.TOTCH, pl.TOT = K, Kb, CO, TOTCH, TOT
    pl.cores = []
    for c in range(P):
        s, d, r, grp, cnt = per_core[c]
        # rank of each edge within its group (edges already group-sorted)
        starts = np.concatenate([[0], np.cumsum(cnt.reshape(-1))])[:-1]
        rank = np.arange(len(s)) - np.repeat(starts, cnt.reshape(-1))
        q = gstart.reshape(-1)[grp] + rank               # slot per edge

        feat_idx = np.zeros(TOT, np.int16)
        er_idx = np.full(TOT, cfg.R_SENT, np.int16)      # pads hit sentinel
        feat_idx[q] = (s >> 1).astype(np.int16)
        er_idx[q] = r.astype(np.int16)

        ind = np.zeros((128, TOT), BF)
        ind_T = np.zeros((128, TOT), BF)
        lane = q % 128
        ch = q // 128
        drel = (d - c * NB) % 128
        ind[lane, ch * 128 + drel] = BF(1.0)
        ind_T[drel, ch * 128 + lane] = BF(1.0)

        def wrap(a):
            w = a.reshape(-1, 16).T                      # [16, TOT/16]
            return np.tile(w, (8, 1)).copy()             # [128, TOT/16]

        core = Plan()
        core.feat_idx = wrap(feat_idx)
        core.er_idx = wrap(er_idx)
        core.ind = ind
        core.ind_T = ind_T
        pl.cores.append(core)
    return pl


def perm_dh(cfg):
    """column permutation: new col d*H+h  <-  old col h*AD+d"""
    c = np.arange(cfg.D)
    d, h = c // cfg.H, c % cfg.H
    return h * cfg.AD + d


# --------------------------------------------------------------------------
# bass program
# --------------------------------------------------------------------------

def build_nc(cfg, pl):
    P, B, NB, NPAD = cfg.P, cfg.B, cfg.NB, cfg.NPAD
    D, H = cfg.D, cfg.H
    TOTCH, TOT = pl.TOTCH, pl.TOT
    TOT16 = TOT // 16
    RG = [list(range(P))]

    nc = bacc.Bacc(None, target_bir_lowering=False, debug=False,
                   num_swdge_queues=4)
    shared = "Shared" if P > 4 else "Local"

    def inp(name, shape, dtype):
        return nc.dram_tensor(name, shape, dtype, kind="ExternalInput")

    # ---- inputs -----------------------------------------------------------
    ent_own = inp("ent_own", [NB, D], F32)
    rel_pad = inp("rel_pad", [cfg.RPAD, D], F32)
    idx_feat = inp("idx_feat", [128, TOT16], I16)
    idx_er = inp("idx_er", [128, TOT16], I16)
    ind_in = inp("ind_in", [128, TOT], BF16)
    indT_in = inp("indT_in", [128, TOT], BF16)
    w_head = inp("w_head", [D, D], BF16)     # col-permuted
    w_tail = inp("w_tail", [D, D], BF16)
    w_ent = inp("w_ent", [D, D], BF16)
    w_rel = inp("w_rel", [D, D], BF16)
    a_h = inp("a_h", [D, H], BF16)           # attn selectors (row-permuted)
    a_t = inp("a_t", [D, H], BF16)
    a_r = inp("a_r", [D, H], BF16)
    w_out = inp("w_out", [D, D], BF16)       # row-permuted
    w1 = inp("w1", [D, 4 * D], BF16)
    w2 = inp("w2", [4 * D, D], BF16)
    g_e = inp("g_e", [128, D], F32)          # replicated LN params
    be_e = inp("be_e", [128, D], F32)
    g_r = inp("g_r", [128, D], F32)
    be_r = inp("be_r", [128, D], F32)
    g_ff = inp("g_ff", [128, D], F32)
    be_ff = inp("be_ff", [128, D], F32)
    b1t = inp("b1t", [128, 8], F32)          # b1 reshaped per o-tile
    b2r = inp("b2r", [128, D], F32)          # b2 replicated
    ident_in = inp("ident_in", [128, 128], BF16)

    out_rows = nc.dram_tensor("out_rows", [NB, D], F32, kind="ExternalOutput")

    # ---- internal DRAM ----------------------------------------------------
    SC0 = 384            # hop-0 augmented row: [feat0 | eh | et | pad]
    er_tbl = nc.dram_tensor("er_tbl", [cfg.RPAD, 128], BF16)
    wid = [SC0] + [D] * (cfg.HOPS - 1)
    slabs = [nc.dram_tensor(f"slab{t}", [NB, wid[t]], BF16)
             for t in range(cfg.HOPS)]
    tbls = [nc.dram_tensor(f"tbl{t}", [NPAD, wid[t]], BF16, addr_space=shared)
            for t in range(cfg.HOPS)]
    feat0s_d = nc.dram_tensor("feat0s_d", [NB, D], BF16)
    ffinal_d = nc.dram_tensor("ffinal_d", [NB, D], BF16)

    with tile.TileContext(nc, num_cores=P) as tc:
        with (
            tc.tile_pool(name="consts", bufs=1) as cp,
            tc.tile_pool(name="work", bufs=3) as wp,
            tc.tile_pool(name="gath", bufs=2) as gp,
            tc.tile_pool(name="pbig", bufs=2, space="PSUM") as pbig,
            tc.tile_pool(name="ptps", bufs=3, space="PSUM") as ptps,
            tc.tile_pool(name="psml", bufs=2, space="PSUM") as psml,
        ):
            from concourse import library_config
            nc.gpsimd.load_library(library_config.mlp)

            # ---- resident constants --------------------------------------
            def load_const(name, dram, shape, dtype):
                t = cp.tile(shape, dtype, name=name)
                nc.sync.dma_start(t[:], dram[:, :])
                return t

            ident = load_const("identc", ident_in, [128, 128], BF16)
            # weights as [128, kt, cols] (k on partitions, k-tiles in free)
            def load_w(name, dram, cols):
                t = cp.tile([128, D // 128, cols], BF16, name=name)
                nc.sync.dma_start(
                    t[:], dram.ap().rearrange("(kt p) c -> p kt c", p=128))
                return t

            whc = load_w("whc", w_head, D)
            wtc = load_w("wtc", w_tail, D)
            wec = load_w("wec", w_ent, D)
            wrc = load_w("wrc", w_rel, D)
            ahc = load_w("ahc", a_h, H)
            atc = load_w("atc", a_t, H)
            arc = load_w("arc", a_r, H)
            woc = load_w("woc", w_out, D)
            w1c = load_w("w1c", w1, 4 * D)
            w2c = cp.tile([128, 4 * D // 128, D], BF16, name="w2c")
            nc.sync.dma_start(
                w2c[:], w2.ap().rearrange("(kt p) c -> p kt c", p=128))
            gec = load_const("gec", g_e, [128, D], F32)
            bec = load_const("bec", be_e, [128, D], F32)
            grc = load_const("grc", g_r, [128, D], F32)
            brc = load_const("brc", be_r, [128, D], F32)
            gfc = load_const("gfc", g_ff, [128, D], F32)
            bfc = load_const("bfc", be_ff, [128, D], F32)
            b1c = load_const("b1c", b1t, [128, 8], F32)
            b2c = load_const("b2c", b2r, [128, D], F32)

            ex_sb = cp.tile([128, TOTCH, 8], BF16, name="ex_sb")
            rden_sb = cp.tile([128, B, 8], F32, name="rden_sb")
            et_own = cp.tile([128, B, 8], BF16, name="et_own")
            eps_t = cp.tile([128, 1], F32, name="eps_t")
            nc.vector.memset(eps_t[:], cfg.EPS)

            # ------------------------------------------------------------------
            # helpers
            # ------------------------------------------------------------------
            def ln(x_f32, gamma, beta, out_t):
                """LayerNorm of [128, D] fp32 tile -> out_t (any dtype)."""
                st = wp.tile([128, 6], F32, name="ln_st", tag="ln_st")
                ag = wp.tile([128, 2], F32, name="ln_ag", tag="ln_ag")
                sd = wp.tile([128, 1], F32, name="ln_sd", tag="ln_sd")
                rv = wp.tile([128, 1], F32, name="ln_rv", tag="ln_rv")
                xc = wp.tile([128, D], F32, name="ln_xc", tag="ln_xc")
                nc.vector.bn_stats(st[:], x_f32)
                nc.vector.bn_aggr(ag[:], st[:])
                nc.scalar.activation(sd[:], ag[:, 1:2], ACTF.Sqrt,
                                     bias=eps_t[:])
                nc.vector.reciprocal(rv[:], sd[:])
                # (x - mu) * rstd
                nc.vector.tensor_scalar(xc[:], x_f32, ag[:, 0:1], rv[:],
                                        ALU.subtract, ALU.mult)
                # * gamma + beta
                nc.vector.scalar_tensor_tensor(
                    xc[:], xc[:], 1.0, gamma, ALU.mult, ALU.mult)
                nc.vector.tensor_tensor(out_t, xc[:], beta, ALU.add)

            def transpose_2(src_bf16, name):
                """[128, D] bf16 -> [128, kt=2, 128] bf16 (transposed tiles)."""
                t = wp.tile([128, D // 128, 128], BF16, name=name, tag="tps_o")
                for k in range(D // 128):
                    ps = ptps.tile([128, 128], BF16, name="tps_ps",
                                   tag="tps")
                    nc.tensor.transpose(
                        ps[:], src_bf16[:, k * 128:(k + 1) * 128], ident[:])
                    nc.scalar.copy(t[:, k, :], ps[:])
                return t

            gather_q = [0]

            def gather(out_t, tbl_view, idx_dram, q0, n, elem, estep, name):
                """dma_gather of n indices starting at padded slot q0."""
                it = gp.tile([128, n // 16], I16, name=name, tag=name)
                nc.sync.dma_start(it[:], idx_dram[:, q0 // 16:(q0 + n) // 16])
                nc.gpsimd.dma_gather(out_t, tbl_view, it[:], n, n, elem,
                                     elem_step=estep, single_packet=False)

            # ------------------------------------------------------------------
            # P0: relation path -> er_tbl  (replicated on every core)
            # ------------------------------------------------------------------
            negt = wp.tile([128, 128], BF16, name="negt", tag="negt")
            nc.vector.memset(negt[:], NEG_BIG)
            for i in range(cfg.RPAD // 128):
                nc.sync.dma_start(er_tbl[i * 128:(i + 1) * 128, :], negt[:])

            for i in range(cfg.RPAD // 128):
                rows0 = i * 128
                nrows = min(cfg.R - rows0, 128) if rows0 < cfg.R else 0
                xr = wp.tile([128, D], F32, name="xr", tag="x_in")
                nc.sync.dma_start(xr[:], rel_pad[rows0:rows0 + 128, :])
                hr = wp.tile([128, D], BF16, name="hr", tag="h_bf")
                ln(xr[:], grc[:], brc[:], hr[:])
                hrt = transpose_2(hr, "hrt")
                # tanh(h @ W_rel) transposed: per o-tile
                tht = wp.tile([128, D // 128, 128], BF16, name="tht", tag="tht")
                for o in range(D // 128):
                    ps = ptps.tile([128, 128], F32, name="proj_ps", tag="tps")
                    for k in range(D // 128):
                        nc.tensor.matmul(
                            ps[:], wrc[:, k, o * 128:(o + 1) * 128],
                            hrt[:, k, :], start=(k == 0), stop=(k == D // 128 - 1))
                    nc.scalar.activation(tht[:, o, :], ps[:], ACTF.Tanh)
                # er_T = A_r^T-contract: [8, 128]
                erp = psml.tile([16, 128], F32, name="erp", tag="sml")
                for o in range(D // 128):
                    nc.tensor.matmul(erp[0:8, :], arc[:, o, :], tht[:, o, :],
                                     start=(o == 0), stop=(o == D // 128 - 1))
                ers = wp.tile([16, 128], BF16, name="ers", tag="ers")
                nc.scalar.copy(ers[0:8, :], erp[0:8, :])
                # transpose [8,128] -> [128, 8]
                ept = ptps.tile([128, 128], BF16, name="ept", tag="tps")
                nc.tensor.transpose(ept[:, 0:8], ers[0:8, :], ident[0:8, 0:8])
                erv = wp.tile([128, 8], BF16, name="erv", tag="erv")
                nc.scalar.copy(erv[:], ept[:, 0:8])
                if nrows > 0:
                    nc.sync.dma_start(
                        er_tbl[rows0:rows0 + nrows, 0:8], erv[0:nrows, :])

            # ------------------------------------------------------------------
            # P1: head — LN, projections, eh/et, feat0
            # ------------------------------------------------------------------
            for i in range(B):
                r0 = i * 128
                xe = wp.tile([128, D], F32, name="xe", tag="x_in")
                nc.sync.dma_start(xe[:], ent_own[r0:r0 + 128, :])
                he = wp.tile([128, D], BF16, name="he", tag="h_bf")
                ln(xe[:], gec[:], bec[:], he[:])
                het = transpose_2(he, "het")

                f0r = wp.tile([128, SC0], BF16, name="f0r", tag="f0r")
                nc.vector.memset(f0r[:], 0.0)
                for (wc, ac, sl) in ((whc, ahc, 0), (wtc, atc, 1)):
                    tht = wp.tile([128, D // 128, 128], BF16, name="thx",
                                  tag="tht")
                    for o in range(D // 128):
                        ps = ptps.tile([128, 128], F32, name="pp", tag="tps")
                        for k in range(D // 128):
                            nc.tensor.matmul(
                                ps[:], wc[:, k, o * 128:(o + 1) * 128],
                                het[:, k, :], start=(k == 0),
                                stop=(k == D // 128 - 1))
                        nc.scalar.activation(tht[:, o, :], ps[:], ACTF.Tanh)
                    ap_ps = psml.tile([16, 128], F32, name="ap_ps",
                                      tag="sml")
                    for o in range(D // 128):
                        nc.tensor.matmul(ap_ps[0:8, :], ac[:, o, :],
                                         tht[:, o, :], start=(o == 0),
                                         stop=(o == D // 128 - 1))
                    aps = wp.tile([8, 128], BF16, name="aps", tag="ers")
                    nc.scalar.copy(aps[:], ap_ps[0:8, :])
                    spt = ptps.tile([128, 128], BF16, name="spt", tag="tps")
                    nc.tensor.transpose(spt[:, 0:8], aps[:], ident[0:8, 0:8])
                    nc.scalar.copy(f0r[:, D + sl * 8:D + sl * 8 + 8],
                                   spt[:, 0:8])
                    if sl == 1:
                        nc.scalar.copy(et_own[:, i, :], spt[:, 0:8])

                # feat0 (no tanh), transposed tiles -> rows
                f0t = wp.tile([128, D // 128, 128], BF16, name="f0t", tag="tht")
                for o in range(D // 128):
                    ps = ptps.tile([128, 128], F32, name="fp", tag="tps")
                    for k in range(D // 128):
                        nc.tensor.matmul(
                            ps[:], wec[:, k, o * 128:(o + 1) * 128],
                            het[:, k, :], start=(k == 0),
                            stop=(k == D // 128 - 1))
                    nc.scalar.copy(f0t[:, o, :], ps[:])
                for o in range(D // 128):
                    ps = ptps.tile([128, 128], BF16, name="fr", tag="tps")
                    nc.tensor.transpose(ps[:], f0t[:, o, :], ident[:])
                    nc.scalar.copy(f0r[:, o * 128:(o + 1) * 128], ps[:])
                nc.sync.dma_start(slabs[0][r0:r0 + 128, :], f0r[:])
                f0s = wp.tile([128, D], BF16, name="f0s", tag="f0s")
                nc.scalar.mul(f0s[:], f0r[:, 0:D], cfg.ALPHA)
                nc.sync.dma_start(feat0s_d[r0:r0 + 128, :], f0s[:])

            # AllGather hop-0 table (feat0 | eh | et)
            nc.gpsimd.collective_compute(
                "AllGather", ALU.bypass, replica_groups=RG,
                ins=[slabs[0].ap().opt()], outs=[tbls[0].ap().opt()])
            er_v = er_tbl.ap()

            # ------------------------------------------------------------------
            # P3: hops
            # ------------------------------------------------------------------
            for t in range(cfg.HOPS):
                W = SC0 if t == 0 else D
                tb_v = tbls[t].ap().rearrange("(n two) c -> n (two c)", two=2)
                tb_even, tb_odd = tb_v[:, 0:W], tb_v[:, W:2 * W]
                out_dram = slabs[t + 1] if t + 1 < cfg.HOPS else ffinal_d
                for b in range(B):
                    co = int(pl.CO[b])
                    k0, k1 = int(pl.K[b, 0]), int(pl.K[b, 1])
                    kb = k0 + k1
                    q0 = co * 128

                    ind_t = gp.tile([128, kb * 128], BF16, name="ind_t",
                                    tag="ind_t")
                    nc.sync.dma_start(ind_t[:, 0:kb * 128],
                                      ind_in[:, q0:q0 + kb * 128])
                    gb = gp.tile([128, kb, W], BF16, name="gb", tag="gb")
                    gather(gb[:, 0:k0, :], tb_even, idx_feat, q0, k0 * 128,
                           W, 2 * W, "ix_f0")
                    gather(gb[:, k0:kb, :], tb_odd, idx_feat, q0 + k0 * 128,
                           k1 * 128, W, 2 * W, "ix_f1")

                    if t == 0:
                        # edge scores: eh (gathered) + et (IndT matmul) + er
                        indT_t = gp.tile([128, kb * 128], BF16, name="indT_t",
                                         tag="indT_t")
                        nc.sync.dma_start(indT_t[:, 0:kb * 128],
                                          indT_in[:, q0:q0 + kb * 128])
                        erg = gp.tile([128, kb, 128], BF16, name="erg",
                                      tag="erg")
                        gather(erg[:, 0:kb, :], er_v, idx_er, q0, kb * 128,
                               128, 128, "ix_er")
                        et_ps = psml.tile([128, kb * 8], F32, name="et_ps",
                                          tag="sml")
                        for ch in range(kb):
                            nc.tensor.matmul(
                                et_ps[:, ch * 8:(ch + 1) * 8],
                                indT_t[:, ch * 128:(ch + 1) * 128],
                                et_own[:, b, :], start=True, stop=True)
                        sc_s = wp.tile([128, kb, 8], F32, name="sc_s",
                                       tag="sc_s")
                        nc.vector.tensor_tensor(
                            sc_s[:, 0:kb, :], gb[:, 0:kb, D:D + 8],
                            et_ps[:].rearrange("p (k e) -> p k e", e=8),
                            ALU.add)
                        nc.vector.tensor_tensor(sc_s[:, 0:kb, :],
                                                sc_s[:, 0:kb, :],
                                                erg[:, :, 0:8], ALU.add)
                        sc_m = wp.tile([128, kb, 8], F32, name="sc_m",
                                       tag="sc_m")
                        nc.vector.tensor_scalar_mul(sc_m[:, 0:kb, :],
                                                    sc_s[:, 0:kb, :],
                                                    cfg.SLOPE)
                        nc.vector.tensor_tensor(sc_s[:, 0:kb, :],
                                                sc_s[:, 0:kb, :],
                                                sc_m[:, 0:kb, :], ALU.max)
                        nc.scalar.activation(ex_sb[:, co:co + kb, :],
                                             sc_s[:, 0:kb, :], ACTF.Exp)
                        den_ps = psml.tile([128, 8], F32, name="den_ps",
                                           tag="sml")
                        for ch in range(kb):
                            nc.tensor.matmul(den_ps[:],
                                             ind_t[:, ch * 128:(ch + 1) * 128],
                                             ex_sb[:, co + ch, :],
                                             start=(ch == 0),
                                             stop=(ch == kb - 1))
                        den_s = wp.tile([128, 8], F32, name="den_s",
                                        tag="den_s")
                        nc.vector.tensor_scalar_add(den_s[:], den_ps[:], 1e-30)
                        rv8 = wp.tile([128, 8], F32, name="rv8", tag="rv8")
                        nc.vector.reciprocal(rv8[:], den_s[:])
                        nc.vector.tensor_scalar_mul(rden_sb[:, b, :], rv8[:],
                                                    1.0 - cfg.ALPHA)

                    # msg = gathered * ex  (broadcast over d within head)
                    gb4 = gb[:, 0:kb, 0:D].rearrange("p k (d h) -> p k d h",
                                                     h=H)
                    exb = (ex_sb[:, co:co + kb, :]
                           .unsqueeze(2)
                           .broadcast_to([128, kb, cfg.AD, H]))
                    nc.vector.tensor_tensor(gb4, gb4, exb, ALU.mult)

                    ps = pbig.tile([128, D], F32, name="agg_ps", tag="agg_ps")
                    for ch in range(kb):
                        nc.tensor.matmul(ps[:],
                                         ind_t[:, ch * 128:(ch + 1) * 128],
                                         gb[:, ch, 0:D],
                                         start=(ch == 0), stop=(ch == kb - 1))
                    # blend: rows = ps * rden  + alpha*feat0
                    f0s_t = wp.tile([128, D], BF16, name="f0s_t", tag="f0s")
                    nc.sync.dma_start(f0s_t[:],
                                      feat0s_d[b * 128:(b + 1) * 128, :])
                    rdb = (rden_sb[:, b, :].unsqueeze(1)
                           .broadcast_to([128, cfg.AD, H]))
                    rows_t = wp.tile([128, D], BF16, name="rows_t", tag="rows")
                    nc.vector.tensor_tensor(
                        rows_t[:].rearrange("p (d h) -> p d h", h=H),
                        ps[:].rearrange("p (d h) -> p d h", h=H),
                        rdb, ALU.mult)
                    nc.vector.tensor_tensor(rows_t[:], rows_t[:], f0s_t[:],
                                            ALU.add)
                    nc.sync.dma_start(out_dram[b * 128:(b + 1) * 128, :],
                                      rows_t[:])
                if t + 1 < cfg.HOPS:
                    nc.gpsimd.collective_compute(
                        "AllGather", ALU.bypass, replica_groups=RG,
                        ins=[slabs[t + 1].ap().opt()],
                        outs=[tbls[t + 1].ap().opt()])

            # ------------------------------------------------------------------
            # P4: tail — W_out, residual, LN, FFN
            # ------------------------------------------------------------------
            for i in range(B):
                r0 = i * 128
                fr = wp.tile([128, D], BF16, name="fr", tag="rows")
                nc.sync.dma_start(fr[:], ffinal_d[r0:r0 + 128, :])
                frt = transpose_2(fr, "frt")
                wo_ps = pbig.tile([128, D], F32, name="wo_ps", tag="agg_ps")
                for k in range(D // 128):
                    nc.tensor.matmul(wo_ps[:], frt[:, k, :], woc[:, k, :],
                                     start=(k == 0), stop=(k == D // 128 - 1))
                xe2 = wp.tile([128, D], F32, name="xe2", tag="x_in")
                nc.sync.dma_start(xe2[:], ent_own[r0:r0 + 128, :])
                rst = wp.tile([128, D], F32, name="rst", tag="rst")
                nc.vector.tensor_tensor(rst[:], wo_ps[:], xe2[:], ALU.add)
                xnb = wp.tile([128, D], BF16, name="xnb", tag="h_bf")
                ln(rst[:], gfc[:], bfc[:], xnb[:])
                xnt = transpose_2(xnb, "xnt")
                # FFN layer 1 (transposed outputs), relu+bias fused
                x2t = wp.tile([128, 4 * D // 128, 128], BF16, name="x2t",
                              tag="x2t")
                for o in range(4 * D // 128):
                    ps1 = ptps.tile([128, 128], F32, name="ps1", tag="tps")
                    for k in range(D // 128):
                        nc.tensor.matmul(
                            ps1[:], w1c[:, k, o * 128:(o + 1) * 128],
                            xnt[:, k, :], start=(k == 0),
                            stop=(k == D // 128 - 1))
                    nc.scalar.activation(x2t[:, o, :], ps1[:], ACTF.Relu,
                                         bias=b1c[:, o:o + 1])
                # FFN layer 2
                ff_ps = pbig.tile([128, D], F32, name="ff_ps", tag="agg_ps")
                for o in range(4 * D // 128):
                    nc.tensor.matmul(ff_ps[:], x2t[:, o, :], w2c[:, o, :],
                                     start=(o == 0),
                                     stop=(o == 4 * D // 128 - 1))
                ot = wp.tile([128, D], F32, name="ot", tag="ot")
                nc.vector.tensor_tensor(ot[:], ff_ps[:], rst[:], ALU.add)
                nc.vector.tensor_tensor(ot[:], ot[:], b2c[:], ALU.add)
                nc.sync.dma_start(out_rows[r0:r0 + 128, :], ot[:])

    from concourse.tile_sem_assignment import PROC_NAME_TO_IDX
    lane_of = {PROC_NAME_TO_IDX[f"DMASW{i}"]: i for i in range(8)}
    for bb in nc.main_func.blocks:
        for inst in bb.instructions:
            if isinstance(inst, mybir.InstDMAGatherAnt):
                lane = lane_of.get(inst.bass_scheduled_proc)
                if lane is not None:
                    inst.queue_num = lane % 4
    nc.finalize()
    return nc


# --------------------------------------------------------------------------
# host orchestration
# --------------------------------------------------------------------------

def make_in_maps(cfg, pl, inputs):
    """Per-core input dicts."""
    P, NB, D, H = cfg.P, cfg.NB, cfg.D, cfg.H
    perm = perm_dh(cfg)

    ent = np.asarray(inputs['ent_embed'], np.float32)
    ent_pad = np.zeros((cfg.NPAD, D), np.float32)
    ent_pad[:cfg.N] = ent
    rel = np.asarray(inputs['rel_embed'], np.float32)
    rel_pad = np.zeros((cfg.RPAD, D), np.float32)
    rel_pad[:cfg.R] = rel

    def repl(v):
        return np.tile(np.asarray(v, np.float32)[None, :], (128, 1)).copy()

    wh = np.asarray(inputs['W_head'], np.float32)[:, perm].astype(BF)
    wt = np.asarray(inputs['W_tail'], np.float32)[:, perm].astype(BF)
    we = np.asarray(inputs['W_ent'], np.float32)[:, perm].astype(BF)
    wr = np.asarray(inputs['W_rel'], np.float32)[:, perm].astype(BF)
    wo = np.asarray(inputs['W_out'], np.float32)[perm, :].astype(BF)
    w1 = np.asarray(inputs['w1'], np.float32).astype(BF)
    w2 = np.asarray(inputs['w2'], np.float32).astype(BF)

    def attn_sel(a):
        a = np.asarray(a, np.float32)          # [H, AD]
        m = np.zeros((D, H), np.float32)
        c = np.arange(D)
        m[c, (c % H)] = a[c % H, c // H]       # row d*H+h holds attn[h, d]
        return m.astype(BF)

    b1 = np.asarray(inputs['b1'], np.float32).reshape(8, 128).T.copy()

    common = dict(
        w_head=wh, w_tail=wt, w_ent=we, w_rel=wr,
        a_h=attn_sel(inputs['attn_h']), a_t=attn_sel(inputs['attn_t']),
        a_r=attn_sel(inputs['attn_r']),
        w_out=wo, w1=w1, w2=w2,
        g_e=repl(inputs['gamma_e']), be_e=repl(inputs['beta_e']),
        g_r=repl(inputs['gamma_r']), be_r=repl(inputs['beta_r']),
        g_ff=repl(inputs['gamma_ff']), be_ff=repl(inputs['beta_ff']),
        b1t=np.ascontiguousarray(b1), b2r=repl(inputs['b2']),
        rel_pad=rel_pad,
        ident_in=np.eye(128, dtype=np.float32).astype(BF),
    )

    in_maps = []
    for c in range(P):
        core = pl.cores[c]
        m = dict(common)
        m['ent_own'] = np.ascontiguousarray(ent_pad[c * NB:(c + 1) * NB])
        m['idx_feat'] = core.feat_idx
        m['idx_er'] = core.er_idx
        m['ind_in'] = core.ind
        m['indT_in'] = core.ind_T
        in_maps.append(m)
    return in_maps


LAST_RESULT = None


def kernel(**inputs) -> np.ndarray:
    global LAST_RESULT
    from concourse.bass_utils import run_bass_kernel_spmd
    cfg = Cfg()
    src = np.asarray(inputs['src']); dst = np.asarray(inputs['dst'])
    eids = np.asarray(inputs['e_ids'])
    pl = plan_edges(cfg, src, dst, eids)
    nc = build_nc(cfg, pl)
    in_maps = make_in_maps(cfg, pl, inputs)
    res = run_bass_kernel_spmd(nc, in_maps, core_ids=list(range(cfg.P)))
    LAST_RESULT = res
    out = np.concatenate([r['out_rows'] for r in res.results], axis=0)
    return out[:cfg.N].astype(np.float32)



# revision 10
# speedup vs baseline: 1.0910x; 1.0910x over previous
"""Trainium2 Bass kernel for a MAGNA-KG message-passing layer (v3).

Problem: N=50000 nodes, E=800000 edges, R=500 relations, D=256 dims,
H=8 heads, 3 PPR hops.  SPMD across 8 NeuronCores.

Sharding (edge parallelism per the hint):
  * nodes range-sharded: core c owns rows [c*NB, (c+1)*NB)
  * edges sharded by owner of dst; within a core edges are grouped by dst
    block (128 nodes), then by table-row parity (int16 gather-index
    workaround), padded to multiples of 128 -> fixed chunk structure baked
    into the SPMD-identical instruction stream
  * hop tables are QUARTER-MAJOR: table rows ordered (quarter, core, row)
    so each hop's AllGather splits into 4 contiguous-slice collectives
    fired as soon as that quarter's dst blocks are done; the last quarter
    is small to shrink the exposed inter-hop bubble
  * the per-edge relation-score gather (er) runs during the head phase
    (where the GPSIMD descriptor generator is otherwise idle) and is
    compacted into a resident [slot, head] SBUF table
  * hop-0 table rows are 768B bf16: [feat0 | eh | pad]; the message
    columns are compacted to a 256-wide tile (ACT) so the attention
    multiply runs as one contiguous DVE pass
  * indicator (segment-sum one-hot) matrices are fp8e4 0/1, streamed
  * LayerNorm centering (x-mu)*rstd runs on the Activation engine
    (Identity with per-partition scale/bias) - the DVE tensor_scalar
    variant measures ~6us/block
  * DVE keeps only long contiguous-run ops; PSUM reads go via ACT copies
  * the output tail (W_out + residual + LN + FFN) is fused per-block into
    hop 2, hiding it under the hop's gather/DMA shadow

kernel(**inputs) takes FULL inputs and returns the FULL [N, 256] output.
"""

import numpy as np
import ml_dtypes

import concourse.bacc as bacc
import concourse.bass as bass
import concourse.mybir as mybir
from concourse import tile

F32 = mybir.dt.float32
BF16 = mybir.dt.bfloat16
FP8 = mybir.dt.float8e4
I16 = mybir.dt.int16
AX = mybir.AxisListType
ALU = mybir.AluOpType
ACTF = mybir.ActivationFunctionType

BF = ml_dtypes.bfloat16
F8 = ml_dtypes.float8_e4m3fn

NEG_BIG = -1.0e9
NQ = 4  # AllGather split factor


class Cfg:
    def __init__(self, N=50000, E=800000, R=500, P=8, HOPS=3,
                 ALPHA=0.15, SLOPE=0.2, EPS=1e-5):
        self.N, self.E, self.R, self.P = N, E, R, P
        self.D, self.H, self.AD = 256, 8, 32
        self.HOPS, self.ALPHA, self.SLOPE, self.EPS = HOPS, ALPHA, SLOPE, EPS
        # blocks of 128 dst nodes per core
        self.B = -(-N // (P * 128))          # ceil
        self.NB = self.B * 128               # nodes per core (padded)
        self.NPAD = P * self.NB              # padded global node count
        self.RPAD = 512                      # relation table rows (>= R+1)
        self.R_SENT = R                      # sentinel row in er table
        # quarter boundaries (in blocks); small last quarter = small bubble
        last = max(1, self.B // 8)
        rest = self.B - last
        base = rest // (NQ - 1)
        extra = rest - base * (NQ - 1)
        self.QB = [base + (1 if q < extra else 0)
                   for q in range(NQ - 1)] + [last]
        self.QB0 = np.concatenate([[0], np.cumsum(self.QB)]).astype(int)
        assert self.QB0[-1] == self.B
        assert self.RPAD >= R + 1


def _cdiv(a, b):
    return -(-a // b)


# --------------------------------------------------------------------------
# host-side planning: edge grouping, gather indices, indicator matrices
# --------------------------------------------------------------------------

class Plan:
    pass


def table_row_of_node(cfg, n):
    """Quarter-major table row for global node id n (vectorized)."""
    c = n // cfg.NB
    r = n % cfg.NB
    blk = r // 128
    q = np.searchsorted(cfg.QB0, blk, side='right') - 1
    q = np.clip(q, 0, NQ - 1)
    rows_q = (np.asarray(cfg.QB) * 128)
    base_rows = np.concatenate([[0], np.cumsum(rows_q * cfg.P)])
    rloc = r - cfg.QB0[q] * 128
    return base_rows[q] + c * rows_q[q] + rloc


def plan_edges(cfg, src, dst, eids):
    """Group edges per core by (dst block, table-row parity); compute shared
    chunk counts; build per-core index/indicator tensors."""
    P, B, NB = cfg.P, cfg.B, cfg.NB
    src = np.asarray(src).astype(np.int64)
    dst = np.asarray(dst).astype(np.int64)
    eids = np.asarray(eids).astype(np.int64)

    trow_all = table_row_of_node(cfg, np.arange(cfg.NPAD))  # node -> table row

    core_of = dst // NB
    per_core = []
    cnts = np.zeros((P, B, 2), np.int64)
    for c in range(P):
        m = core_of == c
        s, d, r = src[m], dst[m], eids[m]
        ts = trow_all[s]                      # table row of src
        blk = (d - c * NB) // 128
        par = ts & 1
        order = np.lexsort((par, blk))
        s, d, r, ts, blk, par = (s[order], d[order], r[order], ts[order],
                                 blk[order], par[order])
        grp = blk * 2 + par
        cnt = np.bincount(grp, minlength=B * 2).reshape(B, 2)
        cnts[c] = cnt
        per_core.append((s, d, r, ts, grp, cnt))

    # shared chunk counts per (block, parity): max over cores, >= 1
    K = np.maximum(_cdiv(cnts, 128).max(axis=0), 1)      # [B, 2]
    Kb = K.sum(axis=1)                                   # [B]
    CO = np.concatenate([[0], np.cumsum(Kb)])            # chunk offset per block
    TOTCH = int(CO[-1])
    TOT = TOTCH * 128

    # padded start position (in edge slots) of each (b, g) group
    gstart = np.zeros((B, 2), np.int64)
    gstart[:, 0] = CO[:-1] * 128
    gstart[:, 1] = CO[:-1] * 128 + K[:, 0] * 128

    pl = Plan()
    pl.K, pl.Kb, pl.CO, pl.TOTCH, pl.TOT = K, Kb, CO, TOTCH, TOT
    pl.cores = []
    for c in range(P):
        s, d, r, ts, grp, cnt = per_core[c]
        # rank of each edge within its group (edges already group-sorted)
        starts = np.concatenate([[0], np.cumsum(cnt.reshape(-1))])[:-1]
        rank = np.arange(len(s)) - np.repeat(starts, cnt.reshape(-1))
        q = gstart.reshape(-1)[grp] + rank               # slot per edge

        feat_idx = np.zeros(TOT, np.int16)
        er_idx = np.full(TOT, cfg.R_SENT, np.int16)      # pads hit sentinel
        feat_idx[q] = (ts >> 1).astype(np.int16)
        er_idx[q] = r.astype(np.int16)

        ind = np.zeros((128, TOT), F8)
        ind_T = np.zeros((128, TOT), F8)
        lane = q % 128
        ch = q // 128
        drel = (d - c * NB) % 128
        ind[lane, ch * 128 + drel] = F8(1.0)
        ind_T[drel, ch * 128 + lane] = F8(1.0)

        def wrap(a):
            w = a.reshape(-1, 16).T                      # [16, TOT/16]
            return np.tile(w, (8, 1)).copy()             # [128, TOT/16]

        core = Plan()
        core.feat_idx = wrap(feat_idx)
        core.er_idx = wrap(er_idx)
        core.ind = ind
        core.ind_T = ind_T
        pl.cores.append(core)
    return pl


def perm_dh(cfg):
    """column permutation: new col d*H+h  <-  old col h*AD+d"""
    c = np.arange(cfg.D)
    d, h = c // cfg.H, c % cfg.H
    return h * cfg.AD + d


# --------------------------------------------------------------------------
# bass program
# --------------------------------------------------------------------------

def build_nc(cfg, pl):
    P, B, NB, NPAD = cfg.P, cfg.B, cfg.NB, cfg.NPAD
    D, H = cfg.D, cfg.H
    TOTCH, TOT = pl.TOTCH, pl.TOT
    TOT16 = TOT // 16
    RG = [list(range(P))]

    nc = bacc.Bacc(None, target_bir_lowering=False, debug=False,
                   num_swdge_queues=4)
    shared = "Shared" if P > 4 else "Local"

    def inp(name, shape, dtype):
        return nc.dram_tensor(name, shape, dtype, kind="ExternalInput")

    # ---- inputs -----------------------------------------------------------
    ent_own = inp("ent_own", [NB, D], F32)
    rel_pad = inp("rel_pad", [cfg.RPAD, D], F32)
    idx_feat = inp("idx_feat", [128, TOT16], I16)
    idx_er = inp("idx_er", [128, TOT16], I16)
    ind_in = inp("ind_in", [128, TOT], FP8)
    indT_in = inp("indT_in", [128, TOT], FP8)
    w_head = inp("w_head", [D, D], BF16)     # col-permuted
    w_tail = inp("w_tail", [D, D], BF16)
    w_ent = inp("w_ent", [D, D], BF16)
    w_rel = inp("w_rel", [D, D], BF16)
    a_h = inp("a_h", [D, H], BF16)           # attn selectors (row-permuted)
    a_t = inp("a_t", [D, H], BF16)
    a_r = inp("a_r", [D, H], BF16)
    w_out = inp("w_out", [D, D], BF16)       # row-permuted
    w1 = inp("w1", [D, 4 * D], BF16)
    w2 = inp("w2", [4 * D, D], BF16)
    g_e = inp("g_e", [128, D], F32)          # replicated LN params
    be_e = inp("be_e", [128, D], F32)
    g_r = inp("g_r", [128, D], F32)
    be_r = inp("be_r", [128, D], F32)
    g_ff = inp("g_ff", [128, D], F32)
    be_ff = inp("be_ff", [128, D], F32)
    b1t = inp("b1t", [128, 8], F32)          # b1 reshaped per o-tile
    b2r = inp("b2r", [128, D], F32)          # b2 replicated
    ident_in = inp("ident_in", [128, 128], BF16)

    out_rows = nc.dram_tensor("out_rows", [NB, D], F32, kind="ExternalOutput")

    # ---- internal DRAM ----------------------------------------------------
    SC0 = 384            # hop-0 row: [feat0 (256) | eh (8) | pad] bf16
    er_tbl = nc.dram_tensor("er_tbl", [cfg.RPAD, 128], BF16)
    slab0 = nc.dram_tensor("slab0", [NB, SC0], BF16)
    tbl0 = nc.dram_tensor("tbl0", [NPAD, SC0], BF16, addr_space=shared)
    slabs = [None] + [nc.dram_tensor(f"slab{t}", [NB, D], BF16)
                      for t in range(1, cfg.HOPS)]
    tbls = [tbl0] + [nc.dram_tensor(f"tbl{t}", [NPAD, D], BF16,
                                    addr_space=shared)
                     for t in range(1, cfg.HOPS)]
    feat0s_d = nc.dram_tensor("feat0s_d", [NB, D], BF16)

    q_rows0 = [int(cfg.QB0[q] * 128) for q in range(NQ + 1)]

    with tile.TileContext(nc, num_cores=P) as tc:
        with (
            tc.tile_pool(name="consts", bufs=1) as cp,
            tc.tile_pool(name="work", bufs=3) as wp,
            tc.tile_pool(name="gath", bufs=3) as gp,
            tc.tile_pool(name="hop0", bufs=2) as h0p,
            tc.tile_pool(name="pbig", bufs=2, space="PSUM") as pbig,
            tc.tile_pool(name="pwf", bufs=2, space="PSUM") as pwf,
            tc.tile_pool(name="ptps", bufs=2, space="PSUM") as ptps,
            tc.tile_pool(name="psml", bufs=2, space="PSUM") as psml,
        ):
            from concourse import library_config
            nc.gpsimd.load_library(library_config.mlp)

            # ---- resident constants --------------------------------------
            def load_const(name, dram, shape, dtype):
                t = cp.tile(shape, dtype, name=name)
                nc.sync.dma_start(t[:], dram[:, :])
                return t

            ident = load_const("identc", ident_in, [128, 128], BF16)
            # weights as [128, kt, cols] (k on partitions, k-tiles in free)
            def load_w(name, dram, cols):
                t = cp.tile([128, D // 128, cols], BF16, name=name)
                nc.sync.dma_start(
                    t[:], dram.ap().rearrange("(kt p) c -> p kt c", p=128))
                return t

            whc = load_w("whc", w_head, D)
            wtc = load_w("wtc", w_tail, D)
            wec = load_w("wec", w_ent, D)
            wrc = load_w("wrc", w_rel, D)
            ahc = load_w("ahc", a_h, H)
            atc = load_w("atc", a_t, H)
            arc = load_w("arc", a_r, H)
            woc = load_w("woc", w_out, D)
            w1c = load_w("w1c", w1, 4 * D)
            w2c = cp.tile([128, 4 * D // 128, D], BF16, name="w2c")
            nc.sync.dma_start(
                w2c[:], w2.ap().rearrange("(kt p) c -> p kt c", p=128))
            gec = load_const("gec", g_e, [128, D], F32)
            bec = load_const("bec", be_e, [128, D], F32)
            grc = load_const("grc", g_r, [128, D], F32)
            brc = load_const("brc", be_r, [128, D], F32)
            gfc = load_const("gfc", g_ff, [128, D], F32)
            bfc = load_const("bfc", be_ff, [128, D], F32)
            b1c = load_const("b1c", b1t, [128, 8], F32)
            b2c = load_const("b2c", b2r, [128, D], F32)

            ex_sb = cp.tile([128, TOTCH, 8], BF16, name="ex_sb")
            er8_sb = cp.tile([128, TOTCH, 8], BF16, name="er8_sb")
            rden_sb = cp.tile([128, B, 8], F32, name="rden_sb")
            et_own = cp.tile([128, B, 8], BF16, name="et_own")
            eps_t = cp.tile([128, 1], F32, name="eps_t")
            nc.vector.memset(eps_t[:], cfg.EPS)

            # ------------------------------------------------------------------
            # helpers
            # ------------------------------------------------------------------
            def ln(x_f32, gamma, beta, out_t):
                """LayerNorm of [128, D] fp32 tile -> out_t (any dtype).

                Centering (x-mu)*rstd runs on ACT (Identity, AP scale+bias)
                - the DVE tensor_scalar variant measures ~6us/block."""
                st = wp.tile([128, 6], F32, name="ln_st", tag="ln_st")
                ag = wp.tile([128, 2], F32, name="ln_ag", tag="ln_ag")
                sd = wp.tile([128, 1], F32, name="ln_sd", tag="ln_sd")
                rv = wp.tile([128, 1], F32, name="ln_rv", tag="ln_rv")
                nm = wp.tile([128, 1], F32, name="ln_nm", tag="ln_nm")
                xc = wp.tile([128, D], F32, name="ln_xc", tag="ln_xc")
                nc.vector.bn_stats(st[:], x_f32)
                nc.vector.bn_aggr(ag[:], st[:])
                nc.scalar.activation(sd[:], ag[:, 1:2], ACTF.Sqrt,
                                     bias=eps_t[:])
                nc.vector.reciprocal(rv[:], sd[:])
                # nm = -mu * rstd
                nc.vector.scalar_tensor_tensor(nm[:], ag[:, 0:1], -1.0, rv[:],
                                               ALU.mult, ALU.mult)
                # xc = x*rstd - mu*rstd
                nc.scalar.activation(xc[:], x_f32, ACTF.Identity,
                                     bias=nm[:], scale=rv[:])
                # * gamma + beta
                nc.vector.scalar_tensor_tensor(
                    xc[:], xc[:], 1.0, gamma, ALU.mult, ALU.mult)
                nc.vector.tensor_tensor(out_t, xc[:], beta, ALU.add)

            def transpose_2(src_bf16, name, dve_copy=False):
                """[128, D] bf16 -> [128, kt=2, 128] bf16 (transposed tiles)."""
                t = wp.tile([128, D // 128, 128], BF16, name=name, tag="tps_o")
                for k in range(D // 128):
                    ps = ptps.tile([128, 128], BF16, name="tps_ps",
                                   tag="tps")
                    nc.tensor.transpose(
                        ps[:], src_bf16[:, k * 128:(k + 1) * 128], ident[:])
                    if dve_copy:
                        nc.vector.tensor_scalar_add(t[:, k, :], ps[:], 0.0)
                    else:
                        nc.scalar.copy(t[:, k, :], ps[:])
                return t

            def gather(out_t, tbl_view, idx_dram, q0, n, elem, estep, name):
                """dma_gather of n indices starting at padded slot q0."""
                it = gp.tile([128, n // 16], I16, name=name, tag=name)
                nc.sync.dma_start(it[:], idx_dram[:, q0 // 16:(q0 + n) // 16])
                nc.gpsimd.dma_gather(out_t, tbl_view, it[:], n, n, elem,
                                     elem_step=estep, single_packet=False)

            ER_HEAD = min(B, max(1, B // 2))

            def er_fetch(bb):
                """er gather for block bb -> compact into resident er8_sb."""
                co_b = int(pl.CO[bb])
                kb_b = int(pl.Kb[bb])
                erg = h0p.tile([128, kb_b, 128], BF16, name="erg", tag="erg")
                gather(erg[:, 0:kb_b, :], er_v, idx_er, co_b * 128,
                       kb_b * 128, 128, 128, "ix_er")
                nc.vector.tensor_scalar_add(er8_sb[:, co_b:co_b + kb_b, :],
                                            erg[:, 0:kb_b, 0:8], 0.0)

            # ------------------------------------------------------------------
            # P0: relation path -> er_tbl  (replicated on every core)
            # ------------------------------------------------------------------
            negt = wp.tile([128, 128], BF16, name="negt", tag="negt")
            nc.vector.memset(negt[:], NEG_BIG)
            for i in range(cfg.RPAD // 128):
                nc.sync.dma_start(er_tbl[i * 128:(i + 1) * 128, :], negt[:])

            for i in range(cfg.RPAD // 128):
                rows0 = i * 128
                nrows = min(cfg.R - rows0, 128) if rows0 < cfg.R else 0
                xr = wp.tile([128, D], F32, name="xr", tag="x_in")
                nc.sync.dma_start(xr[:], rel_pad[rows0:rows0 + 128, :])
                hr = wp.tile([128, D], BF16, name="hr", tag="h_bf")
                ln(xr[:], grc[:], brc[:], hr[:])
                hrt = transpose_2(hr, "hrt", dve_copy=True)
                # tanh(h @ W_rel) transposed: per o-tile
                tht = wp.tile([128, D // 128, 128], BF16, name="tht", tag="tht")
                for o in range(D // 128):
                    ps = ptps.tile([128, 128], F32, name="proj_ps", tag="tps")
                    for k in range(D // 128):
                        nc.tensor.matmul(
                            ps[:], wrc[:, k, o * 128:(o + 1) * 128],
                            hrt[:, k, :], start=(k == 0), stop=(k == D // 128 - 1))
                    nc.scalar.activation(tht[:, o, :], ps[:], ACTF.Tanh)
                # er_T = A_r^T-contract: [8, 128]
                erp = psml.tile([16, 128], F32, name="erp", tag="sml")
                for o in range(D // 128):
                    nc.tensor.matmul(erp[0:8, :], arc[:, o, :], tht[:, o, :],
                                     start=(o == 0), stop=(o == D // 128 - 1))
                ers = wp.tile([16, 128], BF16, name="ers", tag="ers")
                nc.vector.tensor_scalar_add(ers[0:8, :], erp[0:8, :], 0.0)
                # transpose [8,128] -> [128, 8]
                ept = ptps.tile([128, 128], BF16, name="ept", tag="tps")
                nc.tensor.transpose(ept[:, 0:8], ers[0:8, :], ident[0:8, 0:8])
                erv = wp.tile([128, 8], BF16, name="erv", tag="erv")
                nc.vector.tensor_scalar_add(erv[:], ept[:, 0:8], 0.0)
                if nrows > 0:
                    nc.sync.dma_start(
                        er_tbl[rows0:rows0 + nrows, 0:8], erv[0:nrows, :])

            er_v = er_tbl.ap()

            # ------------------------------------------------------------------
            # P1: head — LN, projections, eh/et, feat0; er gathers (GPSIMD
            # idle here) -> resident er8_sb; split AllGather 0
            # ------------------------------------------------------------------
            qnext = 0
            for i in range(B):
                r0 = i * 128
                xe = wp.tile([128, D], F32, name="xe", tag="x_in")
                nc.sync.dma_start(xe[:], ent_own[r0:r0 + 128, :])
                he = wp.tile([128, D], BF16, name="he", tag="h_bf")
                ln(xe[:], gec[:], bec[:], he[:])
                het = transpose_2(he, "het", dve_copy=True)

                # er gather (first ER_HEAD blocks; rest prefetch in hop 0)
                if i < ER_HEAD:
                    er_fetch(i)

                f0r = wp.tile([128, SC0], BF16, name="f0r", tag="f0r")
                nc.vector.memset(f0r[:], 0.0)
                f0r_eh = f0r[:, D:D + 8]
                f0s = wp.tile([128, D], BF16, name="f0s", tag="f0s")
                for (wc, ac, sl) in ((whc, ahc, 0), (wtc, atc, 1)):
                    tht = wp.tile([128, D // 128, 128], BF16, name="thx",
                                  tag="tht")
                    for o in range(D // 128):
                        ps = ptps.tile([128, 128], F32, name="pp", tag="tps")
                        for k in range(D // 128):
                            nc.tensor.matmul(
                                ps[:], wc[:, k, o * 128:(o + 1) * 128],
                                het[:, k, :], start=(k == 0),
                                stop=(k == D // 128 - 1))
                        nc.scalar.activation(tht[:, o, :], ps[:], ACTF.Tanh)
                    ap_ps = psml.tile([16, 128], F32, name="ap_ps",
                                      tag="sml")
                    for o in range(D // 128):
                        nc.tensor.matmul(ap_ps[0:8, :], ac[:, o, :],
                                         tht[:, o, :], start=(o == 0),
                                         stop=(o == D // 128 - 1))
                    aps = wp.tile([8, 128], BF16, name="aps", tag="ers")
                    nc.vector.tensor_scalar_add(aps[:], ap_ps[0:8, :], 0.0)
                    spt = ptps.tile([128, 128], BF16, name="spt", tag="tps")
                    nc.tensor.transpose(spt[:, 0:8], aps[:], ident[0:8, 0:8])
                    if sl == 0:
                        nc.vector.tensor_scalar_add(f0r_eh, spt[:, 0:8], 0.0)
                    else:
                        nc.scalar.copy(et_own[:, i, :], spt[:, 0:8])

                # feat0 (no tanh): fp8 row for the message path + exact
                # alpha-scaled bf16 copy for the blend
                f0t = wp.tile([128, D // 128, 128], BF16, name="f0t", tag="tht")
                for o in range(D // 128):
                    ps = ptps.tile([128, 128], F32, name="fp", tag="tps")
                    for k in range(D // 128):
                        nc.tensor.matmul(
                            ps[:], wec[:, k, o * 128:(o + 1) * 128],
                            het[:, k, :], start=(k == 0),
                            stop=(k == D // 128 - 1))
                    nc.scalar.copy(f0t[:, o, :], ps[:])
                for o in range(D // 128):
                    ps = ptps.tile([128, 128], BF16, name="fr", tag="tps")
                    nc.tensor.transpose(ps[:], f0t[:, o, :], ident[:])
                    nc.vector.tensor_scalar_add(
                        f0r[:, o * 128:(o + 1) * 128], ps[:], 0.0)
                    nc.scalar.mul(f0s[:, o * 128:(o + 1) * 128], ps[:],
                                  cfg.ALPHA)
                nc.sync.dma_start(slab0[r0:r0 + 128, :], f0r[:])
                nc.sync.dma_start(feat0s_d[r0:r0 + 128, :], f0s[:])

                if i + 1 == cfg.QB0[qnext + 1]:
                    r0q, r1q = q_rows0[qnext], q_rows0[qnext + 1]
                    nc.gpsimd.collective_compute(
                        "AllGather", ALU.bypass, replica_groups=RG,
                        ins=[slab0[r0q:r1q, :].opt()],
                        outs=[tbl0[r0q * P:r1q * P, :].opt()])
                    qnext += 1

            # ------------------------------------------------------------------
            # P3: hops (tail fused into the last hop)
            # ------------------------------------------------------------------
            for t in range(cfg.HOPS):
                last = t + 1 == cfg.HOPS
                W = SC0 if t == 0 else D
                tb_v = tbls[t].ap().rearrange("(n two) c -> n (two c)", two=2)
                tb_even, tb_odd = tb_v[:, 0:W], tb_v[:, W:2 * W]
                qnext = 0
                for b in range(B):
                    co = int(pl.CO[b])
                    k0, k1 = int(pl.K[b, 0]), int(pl.K[b, 1])
                    kb = k0 + k1
                    q0 = co * 128

                    ind_t = gp.tile([128, kb * 128], FP8, name="ind_t",
                                    tag="ind_t")
                    nc.sync.dma_start(ind_t[:, 0:kb * 128],
                                      ind_in[:, q0:q0 + kb * 128])

                    if t == 0:
                        if ER_HEAD + b < B:
                            er_fetch(ER_HEAD + b)
                        gbq = gp.tile([128, kb, SC0], BF16, name="gbq",
                                      tag="gb")
                        gather(gbq[:, 0:k0, :], tb_even, idx_feat, q0,
                               k0 * 128, SC0, 2 * SC0, "ix_f0")
                        gather(gbq[:, k0:kb, :], tb_odd, idx_feat,
                               q0 + k0 * 128, k1 * 128, SC0, 2 * SC0, "ix_f1")
                        # compact message columns to 256-pitch (ACT)
                        gb = h0p.tile([128, kb, D], BF16, name="gc", tag="gc")
                        nc.scalar.copy(gb[:, 0:kb, :], gbq[:, 0:kb, 0:D])

                        # scores: eh (exact bf16 in the row) + et + er
                        indT_t = h0p.tile([128, kb * 128], FP8, name="indT_t",
                                          tag="indT_t")
                        nc.sync.dma_start(indT_t[:, 0:kb * 128],
                                          indT_in[:, q0:q0 + kb * 128])
                        et_ps = psml.tile([128, kb * 8], F32, name="et_ps",
                                          tag="sml")
                        for ch in range(kb):
                            nc.tensor.matmul(
                                et_ps[:, ch * 8:(ch + 1) * 8],
                                indT_t[:, ch * 128:(ch + 1) * 128],
                                et_own[:, b, :], start=True, stop=True)
                        eh8 = wp.tile([128, kb, 8], BF16, name="eh8",
                                      tag="eh8")
                        nc.scalar.copy(eh8[:, 0:kb, :],
                                       gbq[:, 0:kb, D:D + 8])
                        et8 = wp.tile([128, kb, 8], BF16, name="et8",
                                      tag="et8")
                        nc.scalar.copy(
                            et8[:, 0:kb, :],
                            et_ps[:].rearrange("p (k e) -> p k e", e=8))
                        sc_s = wp.tile([128, kb, 8], BF16, name="sc_s",
                                       tag="sc_s")
                        nc.vector.tensor_tensor(sc_s[:, 0:kb, :],
                                                eh8[:, 0:kb, :],
                                                et8[:, 0:kb, :], ALU.add)
                        nc.vector.tensor_tensor(sc_s[:, 0:kb, :],
                                                sc_s[:, 0:kb, :],
                                                er8_sb[:, co:co + kb, :],
                                                ALU.add)
                        # leaky relu: max(x, SLOPE*x) in one op
                        nc.vector.scalar_tensor_tensor(
                            sc_s[:, 0:kb, :], sc_s[:, 0:kb, :], cfg.SLOPE,
                            sc_s[:, 0:kb, :], ALU.mult, ALU.max)
                        nc.scalar.activation(ex_sb[:, co:co + kb, :],
                                             sc_s[:, 0:kb, :], ACTF.Exp)
                    else:
                        gb = gp.tile([128, kb, D], BF16, name="gb", tag="gb")
                        gather(gb[:, 0:k0, :], tb_even, idx_feat, q0,
                               k0 * 128, D, 2 * D, "ix_f0")
                        gather(gb[:, k0:kb, :], tb_odd, idx_feat,
                               q0 + k0 * 128, k1 * 128, D, 2 * D, "ix_f1")

                    # msg = gathered * ex  (one contiguous run; broadcast in1)
                    gb4 = gb[:, 0:kb, 0:D].rearrange("p k (d h) -> p k d h",
                                                     h=H)
                    exb = (ex_sb[:, co:co + kb, :]
                           .unsqueeze(2)
                           .broadcast_to([128, kb, cfg.AD, H]))
                    nc.vector.tensor_tensor(gb4, gb4, exb, ALU.mult)

                    ps = pbig.tile([128, D], F32, name="agg_ps", tag="agg_ps")
                    for ch in range(kb):
                        nc.tensor.matmul(ps[:],
                                         ind_t[:, ch * 128:(ch + 1) * 128],
                                         gb[:, ch, 0:D],
                                         start=(ch == 0), stop=(ch == kb - 1))

                    if t == 0:
                        den_ps = psml.tile([128, 8], F32, name="den_ps",
                                           tag="sml")
                        for ch in range(kb):
                            nc.tensor.matmul(den_ps[:],
                                             ind_t[:, ch * 128:(ch + 1) * 128],
                                             ex_sb[:, co + ch, :],
                                             start=(ch == 0),
                                             stop=(ch == kb - 1))
                        den_s = wp.tile([128, 8], F32, name="den_s",
                                        tag="den_s")
                        nc.vector.tensor_scalar_add(den_s[:], den_ps[:],
                                                    1e-30)
                        rv8 = wp.tile([128, 8], F32, name="rv8", tag="rv8")
                        nc.vector.reciprocal(rv8[:], den_s[:])
                        nc.vector.tensor_scalar_mul(rden_sb[:, b, :], rv8[:],
                                                    1.0 - cfg.ALPHA)

                    # blend: rows = ps * rden + alpha*feat0
                    f0s_t = wp.tile([128, D], BF16, name="f0s_t", tag="f0s_t")
                    nc.sync.dma_start(f0s_t[:],
                                      feat0s_d[b * 128:(b + 1) * 128, :])
                    rb = wp.tile([128, D], BF16, name="rb", tag="rows_c")
                    nc.scalar.copy(rb[:], ps[:])
                    rdb = (rden_sb[:, b, :].unsqueeze(1)
                           .broadcast_to([128, cfg.AD, H]))
                    rows_t = wp.tile([128, D], BF16, name="rows_t", tag="rows")
                    nc.vector.tensor_tensor(
                        rows_t[:].rearrange("p (d h) -> p d h", h=H),
                        rb[:].rearrange("p (d h) -> p d h", h=H),
                        rdb, ALU.mult)
                    nc.vector.tensor_tensor(rows_t[:], rows_t[:], f0s_t[:],
                                            ALU.add)

                    if not last:
                        nc.sync.dma_start(
                            slabs[t + 1][b * 128:(b + 1) * 128, :], rows_t[:])
                        if b + 1 == cfg.QB0[qnext + 1]:
                            r0q, r1q = q_rows0[qnext], q_rows0[qnext + 1]
                            nc.gpsimd.collective_compute(
                                "AllGather", ALU.bypass, replica_groups=RG,
                                ins=[slabs[t + 1][r0q:r1q, :].opt()],
                                outs=[tbls[t + 1][r0q * P:r1q * P, :].opt()])
                            qnext += 1
                    else:
                        # ---- fused tail: W_out, residual, LN, FFN ----------
                        r0 = b * 128
                        frt = transpose_2(rows_t, "frt", dve_copy=True)
                        wo_ps = pwf.tile([128, D], F32, name="wo_ps",
                                         tag="wf")
                        for k in range(D // 128):
                            nc.tensor.matmul(wo_ps[:], frt[:, k, :],
                                             woc[:, k, :], start=(k == 0),
                                             stop=(k == D // 128 - 1))
                        xe2 = wp.tile([128, D], F32, name="xe2", tag="x_in")
                        nc.sync.dma_start(xe2[:], ent_own[r0:r0 + 128, :])
                        rst = wp.tile([128, D], F32, name="rst", tag="rst")
                        nc.vector.tensor_tensor(rst[:], wo_ps[:], xe2[:],
                                                ALU.add)
                        xnb = wp.tile([128, D], BF16, name="xnb", tag="h_bf")
                        ln(rst[:], gfc[:], bfc[:], xnb[:])
                        xnt = transpose_2(xnb, "xnt", dve_copy=True)
                        # FFN layer 1 (transposed outputs), relu+bias fused
                        x2t = wp.tile([128, 4 * D // 128, 128], BF16,
                                      name="x2t", tag="x2t")
                        for o in range(4 * D // 128):
                            ps1 = ptps.tile([128, 128], F32, name="ps1",
                                            tag="tps")
                            for k in range(D // 128):
                                nc.tensor.matmul(
                                    ps1[:], w1c[:, k, o * 128:(o + 1) * 128],
                                    xnt[:, k, :], start=(k == 0),
                                    stop=(k == D // 128 - 1))
                            nc.scalar.activation(x2t[:, o, :], ps1[:],
                                                 ACTF.Relu,
                                                 bias=b1c[:, o:o + 1])
                        # FFN layer 2
                        ff_ps = pwf.tile([128, D], F32, name="ff_ps",
                                         tag="wf")
                        for o in range(4 * D // 128):
                            nc.tensor.matmul(ff_ps[:], x2t[:, o, :],
                                             w2c[:, o, :], start=(o == 0),
                                             stop=(o == 4 * D // 128 - 1))
                        ot = wp.tile([128, D], F32, name="ot", tag="ot")
                        nc.vector.tensor_tensor(ot[:], ff_ps[:], rst[:],
                                                ALU.add)
                        nc.vector.tensor_tensor(ot[:], ot[:], b2c[:], ALU.add)
                        nc.sync.dma_start(out_rows[r0:r0 + 128, :], ot[:])

    from concourse.tile_sem_assignment import PROC_NAME_TO_IDX
    lane_of = {PROC_NAME_TO_IDX[f"DMASW{i}"]: i for i in range(8)}
    for bb in nc.main_func.blocks:
        for inst in bb.instructions:
            if isinstance(inst, mybir.InstDMAGatherAnt):
                lane = lane_of.get(inst.bass_scheduled_proc)
                if lane is not None:
                    inst.queue_num = lane % 4
    nc.finalize()
    return nc


# --------------------------------------------------------------------------
# host orchestration
# --------------------------------------------------------------------------

def make_in_maps(cfg, pl, inputs):
    """Per-core input dicts."""
    P, NB, D, H = cfg.P, cfg.NB, cfg.D, cfg.H
    perm = perm_dh(cfg)

    ent = np.asarray(inputs['ent_embed'], np.float32)
    ent_pad = np.zeros((cfg.NPAD, D), np.float32)
    ent_pad[:cfg.N] = ent
    rel = np.asarray(inputs['rel_embed'], np.float32)
    rel_pad = np.zeros((cfg.RPAD, D), np.float32)
    rel_pad[:cfg.R] = rel

    def repl(v):
        return np.tile(np.asarray(v, np.float32)[None, :], (128, 1)).copy()

    wh = np.asarray(inputs['W_head'], np.float32)[:, perm].astype(BF)
    wt = np.asarray(inputs['W_tail'], np.float32)[:, perm].astype(BF)
    we = np.asarray(inputs['W_ent'], np.float32)[:, perm].astype(BF)
    wr = np.asarray(inputs['W_rel'], np.float32)[:, perm].astype(BF)
    wo = np.asarray(inputs['W_out'], np.float32)[perm, :].astype(BF)
    w1 = np.asarray(inputs['w1'], np.float32).astype(BF)
    w2 = np.asarray(inputs['w2'], np.float32).astype(BF)

    def attn_sel(a):
        a = np.asarray(a, np.float32)          # [H, AD]
        m = np.zeros((D, H), np.float32)
        c = np.arange(D)
        m[c, (c % H)] = a[c % H, c // H]       # row d*H+h holds attn[h, d]
        return m.astype(BF)

    b1 = np.asarray(inputs['b1'], np.float32).reshape(8, 128).T.copy()

    common = dict(
        w_head=wh, w_tail=wt, w_ent=we, w_rel=wr,
        a_h=attn_sel(inputs['attn_h']), a_t=attn_sel(inputs['attn_t']),
        a_r=attn_sel(inputs['attn_r']),
        w_out=wo, w1=w1, w2=w2,
        g_e=repl(inputs['gamma_e']), be_e=repl(inputs['beta_e']),
        g_r=repl(inputs['gamma_r']), be_r=repl(inputs['beta_r']),
        g_ff=repl(inputs['gamma_ff']), be_ff=repl(inputs['beta_ff']),
        b1t=np.ascontiguousarray(b1), b2r=repl(inputs['b2']),
        rel_pad=rel_pad,
        ident_in=np.eye(128, dtype=np.float32).astype(BF),
    )

    in_maps = []
    for c in range(P):
        core = pl.cores[c]
        m = dict(common)
        m['ent_own'] = np.ascontiguousarray(ent_pad[c * NB:(c + 1) * NB])
        m['idx_feat'] = core.feat_idx
        m['idx_er'] = core.er_idx
        m['ind_in'] = core.ind
        m['indT_in'] = core.ind_T
        in_maps.append(m)
    return in_maps


LAST_RESULT = None


def kernel(**inputs) -> np.ndarray:
    global LAST_RESULT
    from concourse.bass_utils import run_bass_kernel_spmd
    cfg = Cfg()
    src = np.asarray(inputs['src']); dst = np.asarray(inputs['dst'])
    eids = np.asarray(inputs['e_ids'])
    pl = plan_edges(cfg, src, dst, eids)
    nc = build_nc(cfg, pl)
    in_maps = make_in_maps(cfg, pl, inputs)
    res = run_bass_kernel_spmd(nc, in_maps, core_ids=list(range(cfg.P)))
    LAST_RESULT = res
    out = np.concatenate([r['out_rows'] for r in res.results], axis=0)
    return out[:cfg.N].astype(np.float32)
